# revision 15
# baseline (speedup 1.0000x reference)
"""Trainium2 Bass kernel for nn_L4Attention (GQA attention layer, B=1 T=2048 C=5120,
H=40 Q-heads, 8 KV-heads, D=128, interleaved RoPE, causal).

Sharding: tensor-parallel over 8 cores. Core i owns Q heads [5i, 5i+5), KV head i,
and output columns [640i, 640(i+1)). Attention output yT (head-dim-major, [640, T])
is AllGathered across cores (rank-major concat = full yT [5120, T]) in bf16, then
each core computes its 640 output columns with its Wo row-slice. Host concatenates.

All matmul operands are bf16 (PSUM accumulation stays fp32): bf16 stationaries
enable the PE's fast-weight-load path (fp32r stationary loads are 4x slower)
and halve HBM traffic.

Layout tricks (all transposes are done on host, for free):
 - x is fed as xT [C, T] bf16; weights fed pre-transposed [C, out] bf16.
 - q/k are computed in [d, t] layout; RoPE pairs are made contiguous by permuting
   Wq/Wk rows (evens-then-odds within each head) on host; softmax scale folded
   into Wq.
 - RoPE is applied with partition-offset vector ops (no DMA): with the host
   sign-folded sin table, dst[0:64] = q*cos [0:64] - q*sin [64:128] and
   dst[64:128] = q*cos [64:128] - q*sin [0:64].
 - scores are computed transposed ([s, t]) so softmax sums are along partitions,
   done by an all-ones matmul on the PE which also broadcasts the sum to all
   partitions; exp needs no max-subtraction (tiny scores; masked entries get
   -1e9 bias -> exp underflows to 0 exactly like the reference).
 - v is transposed to [s, d] on-chip via PE-transpose so the PV matmul directly
   produces yT [d, t].
 - q stays in SBUF between stages (no DRAM round trip).
Causality: s-tiles above the diagonal are skipped entirely; diagonal tiles get a
host-built additive bias slice (from attn_bias) and compute only t >= r columns.

Attention is processed per (chunk, head) with a SKEW-deep software pipeline:
scores for s-tile st+SKEW are issued (and exp'd on ACT) before the PV/sum
matmuls of s-tile st, so the exp (and diagonal-tile mask-add) latency hides
under PE work instead of stalling it. Each head normalizes (fast-approx
reciprocal) and stages its yT slice to the AllGather buffer as soon as it
finishes, so the chunk's AllGather fires right after the last head's PV.
Chunks run 3,2,1,0 in both attention and the output projection so gathers
complete in exactly the order the projection consumes them.
"""
import numpy as np
import concourse.bass as bass
import concourse.mybir as mybir
import concourse.tile as tile
from concourse import bacc
from concourse import bass_utils
from concourse.masks import make_identity

N_CORES = 8
T = 2048
C = 5120
H = 40
HKV = 8
D = 128
HQ = H // N_CORES          # 5 q heads per core
P = 128
NCH = 4                    # t-chunks of 512
TCH = T // NCH             # 512
KT = C // P                # 40 contraction tiles
ST = T // P                # 16 s-tiles
XB = 8                     # k-tiles per x-load batch
SKEW = 3                   # attention pipeline depth (s-tiles in flight)
ROPE_BASE = 500000.0
F32 = mybir.dt.float32
BF16 = mybir.dt.bfloat16
MULT = mybir.AluOpType.mult
ADD = mybir.AluOpType.add
SUB = mybir.AluOpType.subtract
EXP = mybir.ActivationFunctionType.Exp

# Attention chunks run smallest-first so the first AllGather fires as early
# as possible: the four AllGathers serialize on the collective engine at
# ~45us each, so the chain must start early for the projection (consuming
# chunks in the same order) to never wait on it.
CHUNK_ORDER = (0, 1, 2, 3)

TRACE = False
TRACE_KW = {}
LAST = {}
_cached_nc = None


def _build_nc():
    nc = bacc.Bacc("TRN2", target_bir_lowering=False, debug=False,
                   enable_asserts=False, num_devices=N_CORES)
    xT = nc.dram_tensor("xT", [C, T], BF16, kind="ExternalInput").ap()
    wqT = nc.dram_tensor("wqT", [C, HQ * D], BF16, kind="ExternalInput").ap()
    wkT = nc.dram_tensor("wkT", [C, D], BF16, kind="ExternalInput").ap()
    wvT = nc.dram_tensor("wvT", [C, D], BF16, kind="ExternalInput").ap()
    woT = nc.dram_tensor("woT", [C, HQ * D], BF16, kind="ExternalInput").ap()
    ccT = nc.dram_tensor("ccT", [P, T], F32, kind="ExternalInput").ap()
    ssT = nc.dram_tensor("ssT", [P, T], F32, kind="ExternalInput").ap()
    maskT = nc.dram_tensor("maskT", [P, NCH, TCH], F32, kind="ExternalInput").ap()
    ones_in = nc.dram_tensor("ones_in", [P, P], BF16, kind="ExternalInput").ap()
    ident_in = nc.dram_tensor("ident_in", [P, P], BF16, kind="ExternalInput").ap()
    outT = nc.dram_tensor("outT", [HQ * D, T], F32, kind="ExternalOutput").ap()

    xT_b = xT.rearrange("(kb xb p) t -> p kb xb t", p=P, xb=XB)   # [128, 5, 8, T]
    xT_b2 = xT.rearrange("(kb xb p) t -> p kb xb t", p=P, xb=2)   # [128, 20, 2, T]
    wqT_r = wqT.rearrange("(kt p) m -> p kt m", p=P)
    wkT_r = wkT.rearrange("(kt p) m -> p kt m", p=P)
    wvT_r = wvT.rearrange("(kt p) m -> p kt m", p=P)
    woT_b = woT.rearrange("(kb xb p) m -> p kb xb m", p=P, xb=XB)  # [128, 5, 8, 640]

    with tile.TileContext(nc) as tc:
        with tc.tile_pool(name="const", bufs=1) as cp, \
             tc.tile_pool(name="dram", bufs=1, space="DRAM") as dramp:
            kT_sb = cp.tile([P, T], BF16)          # rotated k, [d, s]
            v_sb = cp.tile([P, ST, D], BF16)       # v as [s_tile][s, d]
            q_sb = cp.tile([P, HQ, T], BF16)       # rotated q, [d, h, t]
            mask_sb = cp.tile([P, NCH, TCH], F32)
            ones_sb = cp.tile([P, P], BF16)

            yag_in = [dramp.tile([HQ * D, TCH], BF16, tag=f"yi{n}", name=f"yi{n}") for n in range(NCH)]
            yag_out = [dramp.tile([N_CORES * HQ * D, TCH], BF16, tag=f"yo{n}",
                                   name=f"yo{n}", addr_space="Shared")
                       for n in range(NCH)]

            ident = cp.tile([P, P], BF16)
            # wo lives from the start (fits in bf16) but its DMAs are queued on
            # gpsimd behind the stage-1 weight loads: no early-HBM congestion,
            # still resident long before the output projection and clear of
            # the AllGather windows.
            wo_sb = cp.tile([P, KT, HQ * D], BF16)

            # ---------------- stage 1: q/k/v projections + RoPE + v transpose
            with tc.tile_pool(name="w1", bufs=1) as w1p, \
                 tc.tile_pool(name="ps1", bufs=1, space="PSUM") as ps1, \
                 tc.tile_pool(name="s1", bufs=3) as s1:
                wq_sb = w1p.tile([P, KT, HQ * D], BF16)
                wk_sb = w1p.tile([P, KT, D], BF16)
                wv_sb = w1p.tile([P, KT, D], BF16)
                cc_sb = w1p.tile([P, 2, TCH], F32)
                ss_sb = w1p.tile([P, 2, TCH], F32)

                # Stage-1 chunk order puts chunk 0 LAST: attention consumes
                # chunk 0 first, so its unavoidable end-of-stage rope latency
                # coincides with attention's own data dependency instead of
                # adding a second stall (attention's PSUM banks reuse these
                # accumulators' banks, so the handoff waits on rope reads).
                S1_ORDER = (1, 2, 3, 0)
                for idx, n in enumerate(S1_ORDER):
                    first = idx == 0
                    last = idx == NCH - 1
                    tsl = slice(n * TCH, (n + 1) * TCH)
                    qps = [ps1.tile([P, TCH], F32, tag=f"q{h}", name=f"qps{h}", bufs=(2 if h == 0 else 1)) for h in range(HQ)]
                    kps = ps1.tile([P, TCH], F32, tag="kk")
                    vps = ps1.tile([P, TCH], F32, tag="vv")

                    cc_n = cc_sb[:, n % 2, :]
                    ss_n = ss_sb[:, n % 2, :]

                    def rope_mul(src_ps):
                        # phase A of rope: sw = half-swap(src) on ACT (reads
                        # PSUM); tc = src*cos on DVE. The DVE read is what
                        # releases the PSUM bank for reuse.
                        sw_ = s1.tile([P, TCH], F32, tag="rw", bufs=4)
                        tc_ = s1.tile([P, TCH], F32, tag="rc", bufs=4)
                        nc.scalar.copy(sw_[0:64, :], src_ps[64:128, :])
                        nc.scalar.copy(sw_[64:128, :], src_ps[0:64, :])
                        nc.vector.tensor_tensor(tc_[:], src_ps[:], cc_n, MULT)
                        return sw_, tc_

                    def rope_fin(sw_, tc_, dst):
                        # phase B: dst = tc + sw*ss (ss is host-sign-folded)
                        ts_ = s1.tile([P, TCH], F32, tag="rs", bufs=2)
                        nc.vector.tensor_tensor(ts_[:], sw_[:], ss_n, MULT)
                        nc.vector.tensor_tensor(dst, tc_[:], ts_[:], ADD)

                    def rope(src_ps, dst):
                        rope_fin(*rope_mul(src_ps), dst)

                    x_sb = None
                    x0_sb = None
                    for k in range(KT):
                        kb, xb = divmod(k, XB)
                        if first and kb == 0:
                            # first batch: fine-grained x loads so the first
                            # matmul waits on 1/4 of the batch, not all of it
                            if xb % 2 == 0:
                                x0_sb = s1.tile([P, 2, TCH], BF16, tag="x0",
                                                bufs=2, name=f"x0_{xb}")
                                nc.sync.dma_start(x0_sb[:], xT_b2[:, xb // 2, :, tsl])
                            x_view = x0_sb[:, xb % 2, :]
                        else:
                            if xb == 0:
                                x_sb = s1.tile([P, XB, TCH], BF16, tag="x", bufs=2)
                                nc.sync.dma_start(x_sb[:], xT_b[:, kb, :, tsl])
                            x_view = x_sb[:, xb, :]
                        if first:
                            nc.gpsimd.dma_start(wq_sb[:, k, :], wqT_r[:, k, :])
                            nc.gpsimd.dma_start(wk_sb[:, k, :], wkT_r[:, k, :])
                            nc.gpsimd.dma_start(wv_sb[:, k, :], wvT_r[:, k, :])
                        st_, sp_ = (k == 0), (k == KT - 1)
                        for h in range(HQ):
                            nc.tensor.matmul(qps[h][:], wq_sb[:, k, h * D:(h + 1) * D],
                                             x_view, start=st_, stop=sp_)
                        nc.tensor.matmul(kps[:], wk_sb[:, k, :], x_view,
                                         start=st_, stop=sp_)
                        nc.tensor.matmul(vps[:], wv_sb[:, k, :], x_view,
                                         start=st_, stop=sp_)

                    if first:
                        # non-critical constants, issued behind the weight
                        # loads to keep the startup DMA window clear
                        nc.scalar.dma_start(ones_sb[:], ones_in)
                        nc.scalar.dma_start(ident[:], ident_in)
                        nc.scalar.dma_start(mask_sb[:], maskT)
                        nc.gpsimd.dma_start(cc_sb[:, n % 2, :], ccT[:, tsl])
                        nc.gpsimd.dma_start(ss_sb[:, n % 2, :], ssT[:, tsl])
                    if not last:
                        nxt = S1_ORDER[idx + 1]
                        nsl = slice(nxt * TCH, (nxt + 1) * TCH)
                        nc.gpsimd.dma_start(cc_sb[:, nxt % 2, :], ccT[:, nsl])
                        nc.gpsimd.dma_start(ss_sb[:, nxt % 2, :], ssT[:, nsl])
                    if first:
                        for kb in range(KT // XB):
                            nc.gpsimd.dma_start(
                                wo_sb[:, kb * XB:(kb + 1) * XB, :],
                                woT_b[:, kb, :, :])
                    def v_transposes(vtmp):
                        for j in range(4):
                            trp = ps1.tile([P, P], BF16, tag="vv")
                            nc.tensor.transpose(trp[:], vtmp[:, j * P:(j + 1) * P], ident[:])
                            nc.vector.tensor_copy(v_sb[:, n * 4 + j, :], trp[:])

                    if not last:
                        rope(qps[0], q_sb[:, 0, tsl])
                        rope(qps[1], q_sb[:, 1, tsl])
                        vtmp = s1.tile([P, TCH], BF16, tag="vt", bufs=2)
                        nc.scalar.copy(vtmp[:], vps[:])
                        for h in range(2, HQ):
                            rope(qps[h], q_sb[:, h, tsl])
                        rope(kps, kT_sb[:, tsl])
                        v_transposes(vtmp)
                    else:
                        # attention (chunk 0 first) needs k, q0 and v right
                        # away; then release the remaining accumulator banks
                        # early (rope_mul) before finishing those ropes.
                        rope(kps, kT_sb[:, tsl])
                        rope(qps[0], q_sb[:, 0, tsl])
                        vtmp = s1.tile([P, TCH], BF16, tag="vt", bufs=2)
                        nc.scalar.copy(vtmp[:], vps[:])
                        v_transposes(vtmp)
                        parts = [rope_mul(qps[h]) for h in range(1, HQ)]
                        for h, (sw_, tc_) in zip(range(1, HQ), parts):
                            rope_fin(sw_, tc_, q_sb[:, h, tsl])

            # ---------------- stage 2: attention per (chunk, head) + AllGather
            # s3 (the projection's y staging) is opened BEFORE the attention
            # pools so its SBUF addresses don't reuse attention tiles' — the
            # y AllGather-out prefetch DMAs must not wait on attention's last
            # SBUF consumers.
            with tc.tile_pool(name="s3", bufs=1) as s3:
              with tc.tile_pool(name="ps2", bufs=1, space="PSUM") as ps2, \
                 tc.tile_pool(name="s2", bufs=3) as s2:

                def attention_chunk(n):
                    n_st = 4 * (n + 1)          # s-tiles up to diagonal
                    for h in range(HQ):
                        yps = ps2.tile([P, TCH], F32, tag="y", bufs=2,
                                       name=f"yps{n}_{h}")
                        sps = ps2.tile([P, TCH], F32, tag="s", bufs=2,
                                       name=f"sps{n}_{h}")
                        qv = q_sb[:, h, n * TCH:(n + 1) * TCH]
                        scps, exs, esls = {}, {}, {}
                        for i in range(n_st + SKEW):
                            if i < n_st:
                                ssl = slice(i * P, (i + 1) * P)
                                r = (i - 4 * n) * P  # >=0 on diagonal tiles
                                esl = slice(max(r, 0), TCH)
                                esls[i] = esl
                                scp = ps2.tile([P, TCH], F32, tag="sc",
                                               bufs=SKEW + 1, name=f"scp{n}_{h}_{i}")
                                if r >= 0:
                                    # diagonal: only columns t >= r survive
                                    nc.tensor.matmul(
                                        scp[:, r:TCH], kT_sb[:, ssl],
                                        qv[:, r:TCH], start=True, stop=True)
                                    nc.vector.tensor_tensor(
                                        scp[:, r:TCH], scp[:, r:TCH],
                                        mask_sb[:, i - 4 * n, r:TCH], ADD)
                                else:
                                    nc.tensor.matmul(scp[:], kT_sb[:, ssl],
                                                     qv, start=True, stop=True)
                                scps[i] = scp
                                ex = s2.tile([P, TCH], BF16, tag="ex",
                                             bufs=SKEW + 1, name=f"ex{n}_{h}_{i}")
                                nc.scalar.activation(ex[:, esl], scp[:, esl], EXP)
                                exs[i] = ex
                            j = i - SKEW
                            if j >= 0:
                                esl = esls.pop(j)
                                ex = exs.pop(j)
                                scps.pop(j)
                                first, last = (j == 0), (j == n_st - 1)
                                nc.tensor.matmul(yps[:, esl], v_sb[:, j, :],
                                                 ex[:, esl],
                                                 start=first, stop=last)
                                nc.tensor.matmul(sps[:, esl], ones_sb[:],
                                                 ex[:, esl],
                                                 start=first, stop=last)
                        # normalize + stage this head's yT slice right away so
                        # the AllGather can fire as soon as the last head lands
                        inv = s2.tile([P, TCH], F32, tag="inv", bufs=2)
                        nc.vector.reciprocal_approx_fast(out=inv[:], in_=sps[:])
                        yt = s2.tile([P, TCH], BF16, tag="yt", bufs=2,
                                     name=f"yt{n}_{h}")
                        nc.vector.tensor_tensor(yt[:], yps[:], inv[:], MULT)
                        nc.gpsimd.dma_start(
                            yag_in[n].rearrange("(h p) t -> p h t", p=P)[:, h, :],
                            yt[:])
                    nc.gpsimd.collective_compute(
                        "AllGather", mybir.AluOpType.bypass,
                        replica_groups=[list(range(N_CORES))],
                        ins=[yag_in[n].opt()], outs=[yag_out[n].opt()])

                for n in CHUNK_ORDER:
                    attention_chunk(n)

              # ---------------- stage 3: output projection per chunk
              with tc.tile_pool(name="ps3", bufs=1, space="PSUM") as ps3:

                def proj_chunk(n):
                    tsl = slice(n * TCH, (n + 1) * TCH)
                    yfull = yag_out[n].rearrange("(kb xb p) t -> p kb xb t",
                                                 p=P, xb=XB)
                    y_sb = s3.tile([P, KT, TCH], BF16, tag="ys", bufs=2)
                    for kb in range(KT // XB):
                        nc.sync.dma_start(y_sb[:, kb * XB:(kb + 1) * XB, :],
                                          yfull[:, kb, :, :])
                    for m in range(HQ):
                        ops_ = ps3.tile([P, TCH], F32, tag="o", name=f"ops{n}_{m}",
                                        bufs=2)
                        for k in range(KT):
                            nc.tensor.matmul(ops_[:],
                                             wo_sb[:, k, m * D:(m + 1) * D],
                                             y_sb[:, k, :],
                                             start=(k == 0), stop=(k == KT - 1))
                        o_sb = s3.tile([P, TCH], F32, tag="os", bufs=3)
                        nc.vector.tensor_copy(o_sb[:], ops_[:])
                        nc.scalar.dma_start(outT[m * D:(m + 1) * D, tsl], o_sb[:])

                for n in CHUNK_ORDER:
                    proj_chunk(n)

    nc.compile()
    return nc


def _host_inputs(x, Wq, Wk, Wv, Wo, attn_bias):
    bf16 = mybir.dt.np(BF16)
    xT = np.ascontiguousarray(np.asarray(x, np.float32)[0].T).astype(bf16)  # [C, T]
    Wq = np.asarray(Wq, np.float32)
    Wk = np.asarray(Wk, np.float32)
    Wv = np.asarray(Wv, np.float32)
    Wo = np.asarray(Wo, np.float32)
    bias = np.asarray(attn_bias, np.float32)[0, 0]                     # [T, T]

    perm = np.concatenate([np.arange(0, D, 2), np.arange(1, D, 2)])    # evens, odds
    scale = np.float32(1.0 / np.sqrt(D))
    Wq_p = (Wq.reshape(H, D, C)[:, perm, :] * scale).reshape(H * D, C)
    Wk_p = Wk.reshape(HKV, D, C)[:, perm, :]

    # RoPE tables in fp32 (matching the reference)
    inv = (1.0 / (ROPE_BASE ** (np.arange(0, D, 2, dtype=np.float32) / D))).astype(np.float32)
    pos = np.arange(T, dtype=np.float32)
    fr = pos[:, None] * inv[None, :]                                   # [T, 64]
    cosT = np.cos(fr).T.astype(np.float32)                             # [64, T]
    sinT = np.sin(fr).T.astype(np.float32)
    ccT = np.ascontiguousarray(np.concatenate([cosT, cosT], axis=0))   # [128, T]
    ssT = np.ascontiguousarray(np.concatenate([-sinT, sinT], axis=0))  # sign-folded

    # Diagonal-block bias, transposed to [s, r_idx, t]: mask[s, r, t] = bias[t, r*128+s]
    maskT = np.stack([bias[:TCH, r * P:(r + 1) * P].T for r in range(NCH)], axis=1)
    maskT = np.ascontiguousarray(maskT.astype(np.float32))             # [128, 4, 512]

    ones_np = np.ones((P, P), bf16)
    ident_np = np.eye(P, dtype=np.float32).astype(bf16)

    in_maps = []
    for i in range(N_CORES):
        qrows = slice(i * HQ * D, (i + 1) * HQ * D)
        in_maps.append({
            "xT": xT,
            "wqT": np.ascontiguousarray(Wq_p[qrows].T).astype(bf16),
            "wkT": np.ascontiguousarray(Wk_p[i].T).astype(bf16),
            "wvT": np.ascontiguousarray(Wv[i * D:(i + 1) * D].T).astype(bf16),
            "woT": np.ascontiguousarray(Wo[qrows].T).astype(bf16),
            "ccT": ccT,
            "ssT": ssT,
            "maskT": maskT,
            "ones_in": ones_np,
            "ident_in": ident_np,
        })
    return in_maps


def kernel(x, Wq, Wk, Wv, Wo, attn_bias):
    global _cached_nc
    if _cached_nc is None:
        _cached_nc = _build_nc()
    in_maps = _host_inputs(x, Wq, Wk, Wv, Wo, attn_bias)
    res = bass_utils.run_bass_kernel_spmd(
        _cached_nc, in_maps, core_ids=list(range(N_CORES)),
        trace=TRACE, **TRACE_KW)
    LAST["exec_time_ns"] = res.exec_time_ns
    LAST["results"] = res
    out = np.empty((T, C), np.float32)
    for i in range(N_CORES):
        out[:, i * HQ * D:(i + 1) * HQ * D] = np.asarray(res.results[i]["outT"], np.float32).T
    return out.reshape(1, T, C)
